# revision 30
# baseline (speedup 1.0000x reference)
"""B3-spline undecimated wavelet transform (a-trous, 3 levels) on 8 trn2 cores.

kernel(x: [16, 1024, 1024] f32) -> [16, 4, 1024, 1024] f32  ([w1, w2, w3, c3])

Sharding: pure data parallel, batch 16 -> 2 images per NeuronCore.

Per-core kernel: each level's separable dilated 5x5 B3 smoothing is fused
into 5 PSUM-accumulated banded matmuls on the tensor engine:
    y'[h, w] = sum_k W5[k] * (A_d @ y)[h, w + (k-2)*d]
A_d is the banded H-conv matrix with reflect padding folded into top/bottom
blocks; the W-shift is a free-axis offset on the rhs AP; W-reflect comes from
8 mirrored pad columns in SBUF. H uses overlapping 128-row tiles (stride 112)
so each output tile is one K=128 window -> one matmul per tap.

Scheduling: everything is tile-granular so DMA streams continuously instead
of in end-of-level bursts. Queue roles: SP HWDGE ring = per-tile input loads
+ inter-tile seam copies + even-tile output flushes; ACT HWDGE ring = const
loads + PSUM->SBUF evacuation copies; SWDGE (gpsimd) = odd-tile output
flushes. Details w_j = y_{j-1} - y_j run on DVE reading PSUM directly, so
they don't serialize behind the ACT evacuation.
"""
import sys
sys.path.insert(0, "/opt/trn_rl_repo")
import contextlib
import numpy as np
import concourse.bass as bass
import concourse.mybir as mybir
from concourse import bacc
from concourse.tile import TileContext

DT = mybir.dt
F32 = DT.float32
F32R = DT.float32r

H = W = 1024
PAD = 8
PW = W + 2 * PAD
NT = 9
STRIDE = 112
DILS = (1, 2, 4)
W5 = np.array([1.0, 4.0, 6.0, 4.0, 1.0]) / 16.0
TAP_ORDER = (0, 4, 1, 3, 2)
SCALE_OF_TAP = {0: 0, 4: 0, 1: 1, 3: 1, 2: 2}
SCALES = (1.0 / 16.0, 4.0 / 16.0, 6.0 / 16.0)


def tile_geom(t):
    if t == 0:
        return 0, 120, 0
    if t == NT - 1:
        return STRIDE * t + 8, 120, 8
    return STRIDE * t + 8, 112, 8


def build_A(cls, d):
    _, M, lo = tile_geom({"top": 0, "int": 1, "bot": NT - 1}[cls])
    A = np.zeros((128, 128), np.float64)
    for m in range(M):
        for i in range(5):
            if cls == "int":
                k = m + 8 + (i - 2) * d
            elif cls == "top":
                g = m + (i - 2) * d
                k = -g if g < 0 else g
            else:
                g = 904 + m + (i - 2) * d
                k = (2046 - g if g > 1023 else g) - 896
            A[k, lo + m] += W5[i]
    return A


def build(n_img=2, n_cores=8, reps=1, bench=False):
    nc = bacc.Bacc(trn_type="TRN2", target_bir_lowering=False, debug=False,
                   num_devices=n_cores)
    x_d = nc.dram_tensor("x", [n_img, H, W], F32R, kind="ExternalInput")
    if bench:
        o_d = nc.dram_tensor("o_scratch", [n_img, 4, H, W], F32,
                             kind="Internal")
        dummy_d = nc.dram_tensor("out", [1, 64], F32, kind="ExternalOutput")
    else:
        o_d = nc.dram_tensor("o", [n_img, 4, H, W], F32, kind="ExternalOutput")

    # All 36 banded matrices in one DRAM blob, L0's 12 first so the first
    # level's matmuls aren't gated on the full const load.
    keys = []
    for li in range(len(DILS)):
        for cls in ("top", "int", "bot"):
            for si in range(len(SCALES)):
                keys.append((li, cls, si))
    blob = np.zeros((128, len(keys) * 128), np.float32)
    col_of = {}
    for i, (li, cls, si) in enumerate(keys):
        blob[:, 128 * i:128 * (i + 1)] = (
            build_A(cls, DILS[li]) * SCALES[si]).astype(np.float32)
        col_of[(li, cls, si)] = 128 * i
    blob_d = nc.inline_tensor(blob, name="mats")

    with TileContext(nc) as tc:
        ctx = contextlib.ExitStack()
        with ctx:
            consts = ctx.enter_context(tc.tile_pool(name="consts", bufs=1))
            ypool = ctx.enter_context(tc.tile_pool(name="ybuf", bufs=3))
            psum = ctx.enter_context(tc.tile_pool(name="acc", bufs=8, space="PSUM"))
            wstage = ctx.enter_context(tc.tile_pool(name="wstage", bufs=18))

            mat_sb = consts.tile([128, len(keys) * 128], F32R,
                                 tag="mats", name="mats")
            NL0 = 12 * 128
            nc.scalar.dma_start(out=mat_sb[:, 0:NL0],
                                in_=blob_d.ap().bitcast(F32R)[:, 0:NL0])
            nc.scalar.dma_start(out=mat_sb[:, NL0:],
                                in_=blob_d.ap().bitcast(F32R)[:, NL0:])

            def mat(li, cls, si):
                c = col_of[(li, cls, si)]
                return mat_sb[:, c:c + 128]

            def flush_tile(wt, img, ch, t):
                og, M, lo = tile_geom(t)
                nc.gpsimd.dma_start(out=o_d[img, ch, og:og + M, :],
                                    in_=wt[lo:lo + M, :])

            def fill_pads(ybig, t):
                b = PW * t
                nc.vector.tensor_copy(ybig[:, b:b + PAD],
                                      ybig[:, b + 2 * PAD:b + PAD:-1])
                nc.vector.tensor_copy(ybig[:, b + W + PAD:b + W + 2 * PAD],
                                      ybig[:, b + W + PAD - 2:b + W - 2:-1])

            def seams(ybig, t):
                b = PW * t
                if t > 0:
                    nc.sync.dma_start(
                        out=ybig[0:8, b + PAD:b + W + PAD],
                        in_=ybig[112:120, b - PW + PAD:b - PW + W + PAD])
                if t < NT - 1:
                    nc.sync.dma_start(
                        out=ybig[120:128, b + PAD:b + W + PAD],
                        in_=ybig[8:16, b + PW + PAD:b + PW + W + PAD])

            def load_img(img):
                ybig = ypool.tile([128, NT * PW], F32R, tag="ybig", name="ybig")
                for t in range(NT):
                    nc.sync.dma_start(
                        out=ybig[:, PW * t + PAD:PW * t + PAD + W],
                        in_=bass.AP(x_d, (img * H + STRIDE * t) * W,
                                    [[W, 128], [1, W]]))
                    fill_pads(ybig, t)
                return ybig

            def level(img, li, ycur):
                d = DILS[li]
                last = (li == len(DILS) - 1)
                ynext = None
                if not last:
                    ynext = ypool.tile([128, NT * PW], F32R, tag="ybig",
                                       name="ynbig")

                def do_tile(t):
                    og, M, lo = tile_geom(t)
                    cls = "top" if t == 0 else ("bot" if t == NT - 1 else "int")
                    # per-tile staging: the flush DMA reads [lo:lo+M, :] and
                    # the 12-deep pool recycles on a per-tile basis
                    wt = wstage.tile([128, 1024], F32, tag="wt", name="wt")
                    ct = None
                    if last:
                        ct = wstage.tile([128, 1024], F32, tag="wt", name="ct")
                    for c in range(2):
                        col = PAD + 512 * c
                        acc = psum.tile([128, 512], F32, tag="acc", name="acc")
                        for j, i in enumerate(TAP_ORDER):
                            sh = PW * t + col + (i - 2) * d
                            nc.tensor.matmul(
                                acc[:],
                                mat(li, cls, SCALE_OF_TAP[i]),
                                ycur[:, sh:sh + 512],
                                start=(j == 0), stop=(j == 4))
                        if not last:
                            nc.scalar.copy(
                                ynext[:, PW * t + col:PW * t + col + 512],
                                acc[:])
                            y1s = ynext[:, PW * t + col:
                                        PW * t + col + 512].bitcast(F32)
                        else:
                            nc.scalar.copy(ct[:, 512 * c:512 * c + 512],
                                           acc[:])
                            y1s = ct[:, 512 * c:512 * c + 512]
                        y0s = ycur[:, PW * t + col:
                                   PW * t + col + 512].bitcast(F32)
                        # subtract reads the evacuated copy, not PSUM: each
                        # acc bank then has a single reader (the evac) and
                        # frees at the Act engine's pace
                        nc.vector.tensor_tensor(
                            wt[:, 512 * c:512 * c + 512], y0s, y1s,
                            mybir.AluOpType.subtract)
                    flush_tile(wt, img, li, t)
                    if last:
                        flush_tile(ct, img, 3, t)

                for t in range(NT):
                    do_tile(t)
                    if not last:
                        if t >= 2:
                            seams(ynext, t - 2)
                            fill_pads(ynext, t - 2)
                if not last:
                    for t in (NT - 2, NT - 1):
                        seams(ynext, t)
                        fill_pads(ynext, t)
                return ynext

            def run_all():
                # Interleave images at level granularity: the second image's
                # input streams in during the first's L1 compute, so no phase
                # boundary ever waits on an input load (removes the
                # inter-image pipeline bubble).
                if n_img == 2:
                    y0 = load_img(0)
                    y0 = level(0, 0, y0)
                    y1 = load_img(1)
                    y0 = level(0, 1, y0)
                    y1 = level(1, 0, y1)
                    level(0, 2, y0)
                    y1 = level(1, 1, y1)
                    level(1, 2, y1)
                else:
                    for img in range(n_img):
                        y = load_img(img)
                        for li in range(len(DILS)):
                            y = level(img, li, y)

            if bench and reps > 1:
                with tc.For_i(0, reps):
                    run_all()
            else:
                run_all()
            if bench:
                nc.sync.dma_start(out=dummy_d[:], in_=o_d[0, 0, 0:1, 0:64])

    nc.compile()
    return nc


_NC = None


def kernel(x):
    global _NC
    x = np.ascontiguousarray(np.asarray(x), dtype=np.float32)
    B = x.shape[0]
    n_cores = 8
    per = B // n_cores
    if _NC is None:
        _NC = build(n_img=per, n_cores=n_cores)
    from concourse.bass_utils import run_bass_kernel_spmd
    ins = [{"x": np.ascontiguousarray(x[per * c:per * c + per])}
           for c in range(n_cores)]
    res = run_bass_kernel_spmd(_NC, ins, core_ids=list(range(n_cores)))
    return np.concatenate([r["o"] for r in res.results], axis=0)


# revision 33
# speedup vs baseline: 1.1907x; 1.1907x over previous
"""B3-spline undecimated wavelet transform (a-trous, 3 levels) on 8 trn2 cores.

kernel(x: [16, 1024, 1024] f32) -> [16, 4, 1024, 1024] f32  ([w1, w2, w3, c3])

Sharding: pure data parallel, batch 16 -> 2 images per NeuronCore.

Per-core kernel: each level's separable dilated 5x5 B3 smoothing is fused
into 5 PSUM-accumulated banded matmuls on the tensor engine:
    y'[h, w] = sum_k W5[k] * (A_d @ y)[h, w + (k-2)*d]
A_d is the banded H-conv matrix with reflect padding folded into top/bottom
blocks; the W-shift is a free-axis offset on the rhs AP; W-reflect comes from
8 mirrored pad columns in SBUF. H uses overlapping 128-row tiles (stride 112)
so each output tile is one K=128 window -> one matmul per tap.

Scheduling (HW-measured ~98-125us per core vs 206us for the level-burst
original; cost model 142.8us vs 233us):
- Everything is tile-granular so DMA streams continuously instead of in
  end-of-level bursts: per-tile input loads, per-tile [128,1024] w/c staging
  tiles flushed the moment their subtract completes (12-deep pool), seams
  emitted with a 1-tile lag behind the evacuation.
- Queue roles: SP HWDGE ring = input tiles + seam copies only; ACT HWDGE
  ring = const loads + PSUM->SBUF evacuations (engine work); SWDGE (gpsimd)
  = ALL output flushes (A/B-measured faster than splitting them across
  ACT/SWDGE; gpsimd runs no compute so Q7 is free for descriptor gen).
- The two images interleave at level granularity (i0L0 i0L1 i1L0 i0L2 i1L1
  i1L2, ypool bufs=3): the second image's input streams in during the
  first's L1 compute, so no phase boundary waits on an input load.
- The detail subtract w_j = y_{j-1} - y_j reads the EVACUATED copy of y_j,
  not PSUM: each PSUM bank then has a single reader and frees at the Act
  engine's pace (cost model: 155.8 -> 143.1us; PE occupancy 86->91%).
- 36 banded matrices load as 2 merged DMAs (L0's first) on the ACT ring so
  the first matmul fires ~2.5us in.
"""
import sys
sys.path.insert(0, "/opt/trn_rl_repo")
import contextlib
import numpy as np
import concourse.bass as bass
import concourse.mybir as mybir
from concourse import bacc
from concourse.tile import TileContext

DT = mybir.dt
F32 = DT.float32
F32R = DT.float32r

H = W = 1024
PAD = 8
PW = W + 2 * PAD
NT = 9
STRIDE = 112
DILS = (1, 2, 4)
W5 = np.array([1.0, 4.0, 6.0, 4.0, 1.0]) / 16.0
TAP_ORDER = (0, 4, 1, 3, 2)
SCALE_OF_TAP = {0: 0, 4: 0, 1: 1, 3: 1, 2: 2}
SCALES = (1.0 / 16.0, 4.0 / 16.0, 6.0 / 16.0)


def tile_geom(t):
    if t == 0:
        return 0, 120, 0
    if t == NT - 1:
        return STRIDE * t + 8, 120, 8
    return STRIDE * t + 8, 112, 8


def build_A(cls, d):
    _, M, lo = tile_geom({"top": 0, "int": 1, "bot": NT - 1}[cls])
    A = np.zeros((128, 128), np.float64)
    for m in range(M):
        for i in range(5):
            if cls == "int":
                k = m + 8 + (i - 2) * d
            elif cls == "top":
                g = m + (i - 2) * d
                k = -g if g < 0 else g
            else:
                g = 904 + m + (i - 2) * d
                k = (2046 - g if g > 1023 else g) - 896
            A[k, lo + m] += W5[i]
    return A


def build(n_img=2, n_cores=8, reps=1, bench=False,
          wbufs=12, flush_split=False):
    nc = bacc.Bacc(trn_type="TRN2", target_bir_lowering=False, debug=False,
                   num_devices=n_cores)
    x_d = nc.dram_tensor("x", [n_img, H, W], F32R, kind="ExternalInput")
    if bench:
        o_d = nc.dram_tensor("o_scratch", [n_img, 4, H, W], F32,
                             kind="Internal")
        dummy_d = nc.dram_tensor("out", [1, 64], F32, kind="ExternalOutput")
    else:
        o_d = nc.dram_tensor("o", [n_img, 4, H, W], F32, kind="ExternalOutput")

    # All 36 banded matrices in one DRAM blob, L0's 12 first so the first
    # level's matmuls aren't gated on the full const load.
    keys = []
    for li in range(len(DILS)):
        for cls in ("top", "int", "bot"):
            for si in range(len(SCALES)):
                keys.append((li, cls, si))
    blob = np.zeros((128, len(keys) * 128), np.float32)
    col_of = {}
    for i, (li, cls, si) in enumerate(keys):
        blob[:, 128 * i:128 * (i + 1)] = (
            build_A(cls, DILS[li]) * SCALES[si]).astype(np.float32)
        col_of[(li, cls, si)] = 128 * i
    blob_d = nc.inline_tensor(blob, name="mats")

    with TileContext(nc) as tc:
        ctx = contextlib.ExitStack()
        with ctx:
            consts = ctx.enter_context(tc.tile_pool(name="consts", bufs=1))
            ypool = ctx.enter_context(tc.tile_pool(name="ybuf", bufs=3))
            psum = ctx.enter_context(tc.tile_pool(name="acc", bufs=8, space="PSUM"))
            wstage = ctx.enter_context(tc.tile_pool(name="wstage", bufs=wbufs))

            mat_sb = consts.tile([128, len(keys) * 128], F32R,
                                 tag="mats", name="mats")
            NL0 = 12 * 128
            nc.scalar.dma_start(out=mat_sb[:, 0:NL0],
                                in_=blob_d.ap().bitcast(F32R)[:, 0:NL0])
            nc.scalar.dma_start(out=mat_sb[:, NL0:],
                                in_=blob_d.ap().bitcast(F32R)[:, NL0:])

            def mat(li, cls, si):
                c = col_of[(li, cls, si)]
                return mat_sb[:, c:c + 128]

            def flush_tile(wt, img, ch, t):
                og, M, lo = tile_geom(t)
                eng = nc.scalar if (flush_split and t % 2 == 0) else nc.gpsimd
                eng.dma_start(out=o_d[img, ch, og:og + M, :],
                              in_=wt[lo:lo + M, :])

            def fill_pads(ybig, t):
                b = PW * t
                nc.vector.tensor_copy(ybig[:, b:b + PAD],
                                      ybig[:, b + 2 * PAD:b + PAD:-1])
                nc.vector.tensor_copy(ybig[:, b + W + PAD:b + W + 2 * PAD],
                                      ybig[:, b + W + PAD - 2:b + W - 2:-1])

            def seams(ybig, t):
                b = PW * t
                if t > 0:
                    nc.sync.dma_start(
                        out=ybig[0:8, b + PAD:b + W + PAD],
                        in_=ybig[112:120, b - PW + PAD:b - PW + W + PAD])
                if t < NT - 1:
                    nc.sync.dma_start(
                        out=ybig[120:128, b + PAD:b + W + PAD],
                        in_=ybig[8:16, b + PW + PAD:b + PW + W + PAD])

            def load_img(img):
                ybig = ypool.tile([128, NT * PW], F32R, tag="ybig", name="ybig")
                for t in range(NT):
                    nc.sync.dma_start(
                        out=ybig[:, PW * t + PAD:PW * t + PAD + W],
                        in_=bass.AP(x_d, (img * H + STRIDE * t) * W,
                                    [[W, 128], [1, W]]))
                    fill_pads(ybig, t)
                return ybig

            def level(img, li, ycur):
                d = DILS[li]
                last = (li == len(DILS) - 1)
                ynext = None
                if not last:
                    ynext = ypool.tile([128, NT * PW], F32R, tag="ybig",
                                       name="ynbig")

                def do_tile(t):
                    og, M, lo = tile_geom(t)
                    cls = "top" if t == 0 else ("bot" if t == NT - 1 else "int")
                    # per-tile staging: the flush DMA reads [lo:lo+M, :] and
                    # the 12-deep pool recycles on a per-tile basis
                    wt = wstage.tile([128, 1024], F32, tag="wt", name="wt")
                    ct = None
                    if last:
                        ct = wstage.tile([128, 1024], F32, tag="wt", name="ct")
                    for c in range(2):
                        col = PAD + 512 * c
                        acc = psum.tile([128, 512], F32, tag="acc", name="acc")
                        for j, i in enumerate(TAP_ORDER):
                            sh = PW * t + col + (i - 2) * d
                            nc.tensor.matmul(
                                acc[:],
                                mat(li, cls, SCALE_OF_TAP[i]),
                                ycur[:, sh:sh + 512],
                                start=(j == 0), stop=(j == 4))
                        if not last:
                            nc.scalar.copy(
                                ynext[:, PW * t + col:PW * t + col + 512],
                                acc[:])
                            y1s = ynext[:, PW * t + col:
                                        PW * t + col + 512].bitcast(F32)
                        else:
                            nc.scalar.copy(ct[:, 512 * c:512 * c + 512],
                                           acc[:])
                            y1s = ct[:, 512 * c:512 * c + 512]
                        y0s = ycur[:, PW * t + col:
                                   PW * t + col + 512].bitcast(F32)
                        # subtract reads the evacuated copy, not PSUM: each
                        # acc bank then has a single reader (the evac) and
                        # frees at the Act engine's pace
                        nc.vector.tensor_tensor(
                            wt[:, 512 * c:512 * c + 512], y0s, y1s,
                            mybir.AluOpType.subtract)
                    flush_tile(wt, img, li, t)
                    if last:
                        flush_tile(ct, img, 3, t)

                for t in range(NT):
                    do_tile(t)
                    if not last and t >= 1:
                        seams(ynext, t - 1)
                        fill_pads(ynext, t - 1)
                if not last:
                    seams(ynext, NT - 1)
                    fill_pads(ynext, NT - 1)
                return ynext

            def run_all():
                # Interleave images at level granularity: the second image's
                # input streams in during the first's L1 compute, so no phase
                # boundary ever waits on an input load (removes the
                # inter-image pipeline bubble).
                if n_img == 2:
                    y0 = load_img(0)
                    y0 = level(0, 0, y0)
                    y1 = load_img(1)
                    y0 = level(0, 1, y0)
                    y1 = level(1, 0, y1)
                    level(0, 2, y0)
                    y1 = level(1, 1, y1)
                    level(1, 2, y1)
                else:
                    for img in range(n_img):
                        y = load_img(img)
                        for li in range(len(DILS)):
                            y = level(img, li, y)

            if bench and reps > 1:
                with tc.For_i(0, reps):
                    run_all()
            else:
                run_all()
            if bench:
                nc.sync.dma_start(out=dummy_d[:], in_=o_d[0, 0, 0:1, 0:64])

    nc.compile()
    return nc


_NC = None


def kernel(x):
    global _NC
    x = np.ascontiguousarray(np.asarray(x), dtype=np.float32)
    B = x.shape[0]
    n_cores = 8
    per = B // n_cores
    if _NC is None:
        _NC = build(n_img=per, n_cores=n_cores)
    from concourse.bass_utils import run_bass_kernel_spmd
    ins = [{"x": np.ascontiguousarray(x[per * c:per * c + per])}
           for c in range(n_cores)]
    res = run_bass_kernel_spmd(_NC, ins, core_ids=list(range(n_cores)))
    return np.concatenate([r["o"] for r in res.results], axis=0)


# revision 40
# speedup vs baseline: 1.4413x; 1.2104x over previous
"""B3-spline undecimated wavelet transform (a-trous, 3 levels) on 8 trn2 cores.

kernel(x: [16, 1024, 1024] f32) -> [16, 4, 1024, 1024] f32  ([w1, w2, w3, c3])

Sharding: pure data parallel, batch 16 -> 2 images per NeuronCore.

Per-core kernel: each level's separable dilated 5x5 B3 smoothing is fused
into 5 PSUM-accumulated banded matmuls on the tensor engine:
    y'[h, w] = sum_k W5[k] * (A_d @ y)[h, w + (k-2)*d]
A_d is the banded H-conv matrix with reflect padding folded into top/bottom
blocks; the W-shift is a free-axis offset on the rhs AP; W-reflect comes from
8 mirrored pad columns in SBUF. H uses overlapping 128-row tiles (stride 112)
so each output tile is one K=128 window -> one matmul per tap.

Scheduling (HW-measured ~98-125us per core vs 206us for the level-burst
original; cost model 142.8us vs 233us):
- Everything is tile-granular so DMA streams continuously instead of in
  end-of-level bursts: per-tile input loads, per-tile [128,1024] w/c staging
  tiles flushed the moment their subtract completes (12-deep pool), seams
  emitted with a 1-tile lag behind the evacuation.
- Queue roles: SP HWDGE ring = input tiles + seam copies only; ACT HWDGE
  ring = const loads + PSUM->SBUF evacuations (engine work); SWDGE (gpsimd)
  = ALL output flushes (A/B-measured faster than splitting them across
  ACT/SWDGE; gpsimd runs no compute so Q7 is free for descriptor gen).
- The two images interleave at level granularity (i0L0 i0L1 i1L0 i0L2 i1L1
  i1L2, ypool bufs=3): the second image's input streams in during the
  first's L1 compute, so no phase boundary waits on an input load.
- The detail subtract w_j = y_{j-1} - y_j reads the EVACUATED copy of y_j,
  not PSUM: each PSUM bank then has a single reader and frees at the Act
  engine's pace (cost model: 155.8 -> 143.1us; PE occupancy 86->91%).
- 36 banded matrices load as 2 merged DMAs (L0's first) on the ACT ring so
  the first matmul fires ~2.5us in.
"""
import sys
sys.path.insert(0, "/opt/trn_rl_repo")
import contextlib
import numpy as np
import concourse.bass as bass
import concourse.mybir as mybir
from concourse import bacc
from concourse.tile import TileContext

DT = mybir.dt
F32 = DT.float32
F32R = DT.float32r

H = W = 1024
PAD = 8
PW = W + 2 * PAD
NT = 9
STRIDE = 112
DILS = (1, 2, 4)
W5 = np.array([1.0, 4.0, 6.0, 4.0, 1.0]) / 16.0
TAP_ORDER = (0, 4, 1, 3, 2)
SCALE_OF_TAP = {0: 0, 4: 0, 1: 1, 3: 1, 2: 2}
SCALES = (1.0 / 16.0, 4.0 / 16.0, 6.0 / 16.0)


def tile_geom(t):
    if t == 0:
        return 0, 120, 0
    if t == NT - 1:
        return STRIDE * t + 8, 120, 8
    return STRIDE * t + 8, 112, 8


def build_A(cls, d):
    _, M, lo = tile_geom({"top": 0, "int": 1, "bot": NT - 1}[cls])
    A = np.zeros((128, 128), np.float64)
    for m in range(M):
        for i in range(5):
            if cls == "int":
                k = m + 8 + (i - 2) * d
            elif cls == "top":
                g = m + (i - 2) * d
                k = -g if g < 0 else g
            else:
                g = 904 + m + (i - 2) * d
                k = (2046 - g if g > 1023 else g) - 896
            A[k, lo + m] += W5[i]
    return A


def build(n_img=2, n_cores=8, reps=1, bench=False,
          wbufs=12, flush_split=False):
    nc = bacc.Bacc(trn_type="TRN2", target_bir_lowering=False, debug=False,
                   num_devices=n_cores)
    x_d = nc.dram_tensor("x", [n_img, H, W], F32R, kind="ExternalInput")
    if bench:
        o_d = nc.dram_tensor("o_scratch", [n_img, 4, H, W], F32,
                             kind="Internal")
        dummy_d = nc.dram_tensor("out", [1, 64], F32, kind="ExternalOutput")
    else:
        o_d = nc.dram_tensor("o", [n_img, 4, H, W], F32, kind="ExternalOutput")

    # Only the 12 base matrices (A*1/16) travel through HBM; the *4/16 and
    # *6/16 variants are derived on-chip with exact f32 multiplies (entries
    # are j/256 with <=6 significant bits). One DMA per level so L0's
    # weights land in ~0.6us.
    base_keys = []
    for li in range(len(DILS)):
        for cls in ("top", "int", "bot"):
            base_keys.append((li, cls))
    blob = np.zeros((128, len(base_keys) * 128), np.float32)
    base_col = {}
    for i, (li, cls) in enumerate(base_keys):
        blob[:, 128 * i:128 * (i + 1)] = (
            build_A(cls, DILS[li]) * SCALES[0]).astype(np.float32)
        base_col[(li, cls)] = 128 * i
    blob_d = nc.inline_tensor(blob, name="mats")
    # SBUF layout: 12 base cols then 24 derived cols
    col_of = {}
    for i, (li, cls) in enumerate(base_keys):
        col_of[(li, cls, 0)] = 128 * i
    nder = 0
    for li in range(len(DILS)):
        for cls in ("top", "int", "bot"):
            for si in (1, 2):
                col_of[(li, cls, si)] = 128 * (12 + nder)
                nder += 1

    with TileContext(nc) as tc:
        ctx = contextlib.ExitStack()
        with ctx:
            consts = ctx.enter_context(tc.tile_pool(name="consts", bufs=1))
            ypool = ctx.enter_context(tc.tile_pool(name="ybuf", bufs=3))
            psum = ctx.enter_context(tc.tile_pool(name="acc", bufs=8, space="PSUM"))
            wstage = ctx.enter_context(tc.tile_pool(name="wstage", bufs=wbufs))

            mat_sb = consts.tile([128, 36 * 128], F32R,
                                 tag="mats", name="mats")
            for li in range(len(DILS)):
                b = 128 * 3 * li
                # L0's block goes on the SP ring ahead of the input tiles:
                # the ACT stream opens with a ~1.3us LoadActFuncSet that
                # would delay the first-matmul weights
                eng = nc.sync if li == 0 else nc.scalar
                eng.dma_start(
                    out=mat_sb[:, b:b + 3 * 128],
                    in_=blob_d.ap().bitcast(F32R)[:, b:b + 3 * 128])
                for cls in ("top", "int", "bot"):
                    src = mat_sb[:, col_of[(li, cls, 0)]:
                                 col_of[(li, cls, 0)] + 128].bitcast(F32)
                    for si, f in ((1, 4.0), (2, 6.0)):
                        # out dtype stays F32R so the DVE rounds on write
                        # (exact here: entries have <=7 significant bits)
                        dst = mat_sb[:, col_of[(li, cls, si)]:
                                     col_of[(li, cls, si)] + 128]
                        nc.vector.tensor_scalar_mul(dst, src, f)

            def mat(li, cls, si):
                c = col_of[(li, cls, si)]
                return mat_sb[:, c:c + 128]

            # pre-warm the PE p-state during the input load: a dozen dummy
            # matmuls on the (already resident) L0 weights ramp HAM before
            # the first real matmul
            warm = psum.tile([128, 512], F32, tag="acc", name="warm")
            for _ in range(12):
                nc.tensor.matmul(warm[:, 0:384], mat_sb[:, 0:128],
                                 mat_sb[:, 0:384], start=True, stop=True)

            def flush_tile(wt, img, ch, t):
                og, M, lo = tile_geom(t)
                eng = nc.scalar if (flush_split and t % 2 == 0) else nc.gpsimd
                eng.dma_start(out=o_d[img, ch, og:og + M, :],
                              in_=wt[lo:lo + M, :])

            def fill_pads(ybig, t):
                b = PW * t
                nc.vector.tensor_copy(ybig[:, b:b + PAD],
                                      ybig[:, b + 2 * PAD:b + PAD:-1])
                nc.vector.tensor_copy(ybig[:, b + W + PAD:b + W + 2 * PAD],
                                      ybig[:, b + W + PAD - 2:b + W - 2:-1])

            def seams(ybig, t):
                b = PW * t
                if t > 0:
                    nc.sync.dma_start(
                        out=ybig[0:8, b + PAD:b + W + PAD],
                        in_=ybig[112:120, b - PW + PAD:b - PW + W + PAD])
                if t < NT - 1:
                    nc.sync.dma_start(
                        out=ybig[120:128, b + PAD:b + W + PAD],
                        in_=ybig[8:16, b + PW + PAD:b + PW + W + PAD])

            def load_img(img):
                ybig = ypool.tile([128, NT * PW], F32R, tag="ybig", name="ybig")
                for t in range(NT):
                    nc.sync.dma_start(
                        out=ybig[:, PW * t + PAD:PW * t + PAD + W],
                        in_=bass.AP(x_d, (img * H + STRIDE * t) * W,
                                    [[W, 128], [1, W]]))
                    fill_pads(ybig, t)
                return ybig

            def level(img, li, ycur):
                d = DILS[li]
                last = (li == len(DILS) - 1)
                ynext = None
                if not last:
                    ynext = ypool.tile([128, NT * PW], F32R, tag="ybig",
                                       name="ynbig")

                def do_tile(t):
                    og, M, lo = tile_geom(t)
                    cls = "top" if t == 0 else ("bot" if t == NT - 1 else "int")
                    # per-tile staging: the flush DMA reads [lo:lo+M, :] and
                    # the 12-deep pool recycles on a per-tile basis
                    wt = wstage.tile([128, 1024], F32, tag="wt", name="wt")
                    ct = None
                    if last:
                        ct = wstage.tile([128, 1024], F32, tag="wt", name="ct")
                    for c in range(2):
                        col = PAD + 512 * c
                        acc = psum.tile([128, 512], F32, tag="acc", name="acc")
                        for j, i in enumerate(TAP_ORDER):
                            sh = PW * t + col + (i - 2) * d
                            nc.tensor.matmul(
                                acc[:],
                                mat(li, cls, SCALE_OF_TAP[i]),
                                ycur[:, sh:sh + 512],
                                start=(j == 0), stop=(j == 4))
                        if not last:
                            nc.scalar.copy(
                                ynext[:, PW * t + col:PW * t + col + 512],
                                acc[:])
                            y1s = ynext[:, PW * t + col:
                                        PW * t + col + 512].bitcast(F32)
                        else:
                            nc.scalar.copy(ct[:, 512 * c:512 * c + 512],
                                           acc[:])
                            y1s = ct[:, 512 * c:512 * c + 512]
                        y0s = ycur[:, PW * t + col:
                                   PW * t + col + 512].bitcast(F32)
                        # subtract reads the evacuated copy, not PSUM: each
                        # acc bank then has a single reader (the evac) and
                        # frees at the Act engine's pace
                        nc.vector.tensor_tensor(
                            wt[:, 512 * c:512 * c + 512], y0s, y1s,
                            mybir.AluOpType.subtract)
                    flush_tile(wt, img, li, t)
                    if last:
                        flush_tile(ct, img, 3, t)

                for t in range(NT):
                    do_tile(t)
                    if not last and t >= 1:
                        seams(ynext, t - 1)
                        fill_pads(ynext, t - 1)
                if not last:
                    seams(ynext, NT - 1)
                    fill_pads(ynext, NT - 1)
                return ynext

            def run_all():
                # Interleave images at level granularity: the second image's
                # input streams in during the first's L1 compute, so no phase
                # boundary ever waits on an input load (removes the
                # inter-image pipeline bubble).
                if n_img == 2:
                    y0 = load_img(0)
                    y0 = level(0, 0, y0)
                    y1 = load_img(1)
                    y0 = level(0, 1, y0)
                    y1 = level(1, 0, y1)
                    level(0, 2, y0)
                    y1 = level(1, 1, y1)
                    level(1, 2, y1)
                else:
                    for img in range(n_img):
                        y = load_img(img)
                        for li in range(len(DILS)):
                            y = level(img, li, y)

            if bench and reps > 1:
                with tc.For_i(0, reps):
                    run_all()
            else:
                run_all()
            if bench:
                nc.sync.dma_start(out=dummy_d[:], in_=o_d[0, 0, 0:1, 0:64])

    nc.compile()
    return nc


_NC = None


def kernel(x):
    global _NC
    x = np.ascontiguousarray(np.asarray(x), dtype=np.float32)
    B = x.shape[0]
    n_cores = 8
    per = B // n_cores
    if _NC is None:
        _NC = build(n_img=per, n_cores=n_cores)
    from concourse.bass_utils import run_bass_kernel_spmd
    ins = [{"x": np.ascontiguousarray(x[per * c:per * c + per])}
           for c in range(n_cores)]
    res = run_bass_kernel_spmd(_NC, ins, core_ids=list(range(n_cores)))
    return np.concatenate([r["o"] for r in res.results], axis=0)
